# revision 1
# baseline (speedup 1.0000x reference)
"""Trainium2 Bass kernel for nn_Base_Filter (depthwise 7x7 conv + weight-norm +
1x1 projection residual + leaky-decay-relu), sharded over K=1024 channels
across 8 NeuronCores.

Math (folded on host):
  y      = x*(1+w_p) + b_p                       (per-channel affine)
  w_eff  = g * v / ||v||_F                       (weight norm, per channel)
  z      = depthwise_conv7x7_valid(y, w_eff)
  out    = where(z>0, 0.9*z, 0.01*z)

Linearity fold: z = conv(x, w_eff)*(1+w_p) + b_p*sum(w_eff), so with
  w2 = 0.9*(1+w_p)*w_eff,  c2 = 0.9*b_p*sum(w_eff)
we get  out = lrelu(conv(x, w2) + c2, alpha=1/90)  elementwise.

Device kernel (per core, 128 channels on 128 partitions):
  - 49-tap accumulation split three ways (N_PE/N_ACT/N_DVE = 32/12/5):
      TensorE: diagonal-lhsT float32r matmuls (full rate, 1 cycle/row)
               accumulating in PSUM, two 2-row groups per 2-bank tile;
      ScalarE: per-partition-scale multiplies into bf16 temps;
      VectorE: folds the bf16 temps at 2x, runs scalar_tensor_tensor taps,
               and merges the PSUM total.
  - ScalarE applies Lrelu(+bias c2) while evacuating to SBUF.
  - All DMA is contiguous per partition (host pre-transposes x to
    channel-major [1024, 256, 256] and post-transposes the output).
TimelineSim (calibrated cost model): ~897 us/core; engines ~97%/90%/85%
busy (PE/DVE/ACT). HW-verified rel err 2.4e-3 (scale-rel absmax 6.3e-3).
"""

import os
import numpy as np

A = 256
B = 256
R = 32
C = 32
K = 1024
KS = 7
NCORES = 8
P = 128          # channels per core = partitions
AO = A - KS + 1  # 250
BO = B - KS + 1  # 250

H = 24           # output rows per strip
TR = 4           # output rows per PSUM tile (2 banks; matmuls go per 2-row half)
BP = 256         # padded row pitch in PSUM so each 2-row half sits in one bank

# Tap split between TensorE / ScalarE(+VectorE fold) / VectorE (49 total).
N_PE = int(os.environ.get("KRN_N_PE", "32"))
N_ACT = int(os.environ.get("KRN_N_ACT", "12"))
N_DVE = KS * KS - N_PE - N_ACT

_COMPILED = {}
LAST_RESULTS = None  # BassKernelResults of the most recent run (for test.py)


def _build_nc():
    import concourse.bacc as bacc
    import concourse.mybir as mybir
    import concourse.tile as tile

    f32 = mybir.dt.float32
    nc = bacc.Bacc("TRN2", target_bir_lowering=False, debug=False, num_devices=NCORES)

    f32r = mybir.dt.float32r
    x_d = nc.declare_dram_parameter("x", [P, A, B], f32r, isOutput=False)
    dg_d = nc.declare_dram_parameter("dg", [P, max(N_PE, 1), P], f32r, isOutput=False)
    wv_d = nc.declare_dram_parameter(
        "wv", [P, max(N_ACT + N_DVE, 1)], f32, isOutput=False
    )
    c2_d = nc.declare_dram_parameter("c2", [P, 1], f32, isOutput=False)
    out_d = nc.declare_dram_parameter("out", [P, AO, BO], f32, isOutput=True)

    bf16 = mybir.dt.bfloat16
    taps = [(di, dj) for di in range(KS) for dj in range(KS)]
    pe_taps = taps[:N_PE]
    act_taps = taps[N_PE : N_PE + N_ACT]
    dve_taps = taps[N_PE + N_ACT :]

    with tile.TileContext(nc) as tc:
        from contextlib import ExitStack

        with ExitStack() as ctx:
            const = ctx.enter_context(tc.tile_pool(name="const", bufs=1))
            xpool = ctx.enter_context(tc.tile_pool(name="x", bufs=2))
            opool = ctx.enter_context(tc.tile_pool(name="o", bufs=2))
            apool = ctx.enter_context(tc.tile_pool(name="acc", bufs=3))
            bpool = ctx.enter_context(tc.tile_pool(name="accb", bufs=3))
            tpool = ctx.enter_context(tc.tile_pool(name="tmp", bufs=max(N_ACT + 3, 4)))
            ppool = ctx.enter_context(tc.tile_pool(name="ps", bufs=4, space="PSUM"))

            dg_sb = const.tile([P, max(N_PE, 1), P], f32r)
            nc.sync.dma_start(dg_sb[:], dg_d[:])
            wv_sb = const.tile([P, max(N_ACT + N_DVE, 1)], f32)
            nc.sync.dma_start(wv_sb[:], wv_d[:])
            c2_sb = const.tile([P, 1], f32)
            nc.sync.dma_start(c2_sb[:], c2_d[:])

            row0 = 0
            while row0 < AO:
                rows = min(H, AO - row0)
                in_rows = rows + KS - 1
                xs = xpool.tile([P, in_rows, B], f32r, tag="xs")
                nc.sync.dma_start(xs[:], x_d[:, row0 : row0 + in_rows, :])
                outs = opool.tile([P, rows, BO], f32, tag="outs")

                o0 = 0
                while o0 < rows:
                    tr = min(TR, rows - o0)
                    ps = ppool.tile([P, TR, BP], f32, tag="ps")
                    for h in range(0, tr, 2):
                        hr = min(2, tr - h)
                        out_ap = ps[:, h : h + hr, 0:BO]
                        for i, (di, dj) in enumerate(pe_taps):
                            rhs = xs[:, o0 + h + di : o0 + h + di + hr, dj : dj + BO]
                            # float32r: full-rate (1 cycle/row) fp32 matmul
                            nc.tensor.matmul(
                                out_ap,
                                dg_sb[:, i, :],
                                rhs,
                                start=(i == 0),
                                stop=(i == len(pe_taps) - 1),
                            )
                    ps_ap = ps[:, 0:tr, 0:BO]

                    # ScalarE-assist taps: tmp_m = x_win * w  (bf16), folded
                    # pairwise on VectorE at bf16 2x rate into accb.
                    accb_ap = None
                    if N_ACT > 0:
                        accb = bpool.tile([P, TR, BP], bf16, tag="accb")
                        accb_ap = accb[:, 0:tr, 0:BO]
                        tmps = []
                        for m, (di, dj) in enumerate(act_taps):
                            rhs = xs[
                                :, o0 + di : o0 + di + tr, dj : dj + BO
                            ].bitcast(f32)
                            tmp = tpool.tile([P, TR, BP], bf16, tag="tmp")
                            nc.scalar.mul(
                                tmp[:, 0:tr, 0:BO], rhs, wv_sb[:, m : m + 1]
                            )
                            tmps.append(tmp[:, 0:tr, 0:BO])
                        nc.vector.tensor_tensor(
                            accb_ap, tmps[0], tmps[1], mybir.AluOpType.add
                        )
                        for m in range(2, N_ACT):
                            nc.vector.tensor_tensor(
                                accb_ap, accb_ap, tmps[m], mybir.AluOpType.add
                            )

                    # VectorE stt taps first (independent of PSUM), then
                    # fold accb and the PSUM total at the end.
                    acc = apool.tile([P, TR, BO], f32, tag="acc")
                    acc_ap = acc[:, 0:tr, :]
                    for j, (di, dj) in enumerate(dve_taps):
                        rhs = xs[
                            :, o0 + di : o0 + di + tr, dj : dj + BO
                        ].bitcast(f32)
                        if j == 0:
                            nc.vector.tensor_scalar(
                                acc_ap,
                                rhs,
                                wv_sb[:, N_ACT : N_ACT + 1],
                                None,
                                mybir.AluOpType.mult,
                            )
                        else:
                            nc.vector.scalar_tensor_tensor(
                                acc_ap,
                                rhs,
                                wv_sb[:, N_ACT + j : N_ACT + j + 1],
                                acc_ap,
                                mybir.AluOpType.mult,
                                mybir.AluOpType.add,
                            )
                    if accb_ap is not None:
                        nc.vector.tensor_tensor(
                            acc_ap, acc_ap, accb_ap, mybir.AluOpType.add
                        )
                    nc.vector.tensor_tensor(
                        acc_ap, acc_ap, ps_ap, mybir.AluOpType.add
                    )
                    src = acc_ap
                    # out = lrelu(src + c2), alpha = 0.01/0.9
                    nc.scalar.activation(
                        outs[:, o0 : o0 + tr, :],
                        src,
                        mybir.ActivationFunctionType.Lrelu,
                        bias=c2_sb[:, 0:1],
                        scale=1.0,
                        alpha=0.01 / 0.9,
                    )
                    o0 += tr

                nc.sync.dma_start(out_d[:, row0 : row0 + rows, :], outs[:])
                row0 += rows

    nc.compile()
    return nc


def _prep_weights(w_p, b_p, v, g):
    v = v.astype(np.float32)
    v_norm = np.sqrt((v * v).sum(axis=(1, 2), keepdims=True))
    w_eff = g[:, None, None].astype(np.float32) * v / v_norm          # [K,7,7]
    w2 = 0.9 * (1.0 + w_p)[:, None, None].astype(np.float32) * w_eff  # [K,7,7]
    c2 = (0.9 * b_p.astype(np.float32) * w_eff.sum(axis=(1, 2)))      # [K]
    return w2.astype(np.float32), c2.astype(np.float32)


def kernel(x, w_p, b_p, v, g):
    global LAST_RESULTS
    from concourse.bass_utils import run_bass_kernel_spmd

    x = np.asarray(x, dtype=np.float32)
    w2, c2 = _prep_weights(
        np.asarray(w_p, np.float32),
        np.asarray(b_p, np.float32),
        np.asarray(v, np.float32),
        np.asarray(g, np.float32),
    )

    # channel-major x: [K, A, B], k = r*C + c (matches reference's kernel_index)
    x_t = np.ascontiguousarray(x.transpose(2, 3, 0, 1).reshape(K, A, B))

    taps = [(di, dj) for di in range(KS) for dj in range(KS)]
    in_maps = []
    ar = np.arange(P)
    for core in range(NCORES):
        sl = slice(core * P, (core + 1) * P)
        w2c = w2[sl]  # [P,7,7]
        dg = np.zeros((max(N_PE, 1), P, P), dtype=np.float32)
        for i, (di, dj) in enumerate(taps[:N_PE]):
            dg[i, ar, ar] = w2c[:, di, dj]
        # SBUF layout [P, N_PE, P]: dg_sb[p, t, m] = dg[t, p, m]
        dg_sb = np.ascontiguousarray(dg.transpose(1, 0, 2))
        wv = np.zeros((P, max(N_ACT + N_DVE, 1)), dtype=np.float32)
        for j, (di, dj) in enumerate(taps[N_PE:]):
            wv[:, j] = w2c[:, di, dj]
        in_maps.append(
            {
                "x": np.ascontiguousarray(x_t[sl]),
                "dg": dg_sb,
                "wv": wv,
                "c2": np.ascontiguousarray(c2[sl][:, None]),
            }
        )

    assert N_DVE >= 1 and N_PE >= 1
    key = ("v1", N_PE, N_ACT)
    if key not in _COMPILED:
        _COMPILED[key] = _build_nc()
    nc = _COMPILED[key]

    trace = os.environ.get("KRN_TRACE", "0") == "1"
    res = run_bass_kernel_spmd(nc, in_maps, list(range(NCORES)), trace=trace)
    LAST_RESULTS = res

    out_full = np.empty((K, AO, BO), dtype=np.float32)
    for core in range(NCORES):
        out_full[core * P : (core + 1) * P] = res.results[core]["out"]

    # [K, AO, BO] -> [AO, BO, R, C]
    return np.ascontiguousarray(
        out_full.reshape(R, C, AO, BO).transpose(2, 3, 0, 1)
    )


if __name__ == "__main__":
    rng = np.random.default_rng(0)
    xs = rng.standard_normal((A, B, R, C), dtype=np.float32)
    out = kernel(
        xs,
        rng.standard_normal(K).astype(np.float32) * 0.1,
        rng.standard_normal(K).astype(np.float32) * 0.1,
        rng.standard_normal((K, KS, KS)).astype(np.float32),
        rng.standard_normal(K).astype(np.float32),
    )
    print(out.shape, out.dtype)



# revision 4
# speedup vs baseline: 4.0112x; 4.0112x over previous
"""Trainium2 Bass kernel for nn_Base_Filter (depthwise 7x7 conv + weight-norm +
1x1 projection residual + leaky-decay-relu), sharded over K=1024 channels
across 8 NeuronCores (128 channels per core).

Math (folded on host):
  y      = x*(1+w_p) + b_p                       (per-channel affine)
  w_eff  = g * v / ||v||_F                       (weight norm, per channel)
  z      = depthwise_conv7x7_valid(y, w_eff)
  out    = where(z>0, 0.9*z, 0.01*z)

Linearity fold: with w2 = 0.9*(1+w_p)*w_eff, c2 = 0.9*b_p*sum(w_eff):
  out = lrelu(conv(x, w2) + c2, alpha=1/90).

Device kernel (banded-matmul formulation, per core):
  For each channel, put IMAGE ROWS on the 128 SBUF partitions: partition p
  holds rows p and 128+p (two "halves" h=0/1).  The 7 vertical taps (di) are
  folded into a banded stationary operand lhsT[p, j] = w2[p-j, dj] (built on
  host, bf16), so ONE matmul computes, for all 122 output rows j and both
  halves, the di-contraction at a fixed horizontal tap dj:
     psum[j, (h,c)] += sum_p lhsT[p,j] * x[p, (h, c+dj)]
  The 7 horizontal taps (dj) are just free-axis offsets into the SAME x tile
  (7 accumulating matmuls, free size 2*250=500 each).  This does 7 taps per
  pass over the outputs vs 1 tap/pass for diagonal-matmul schemes.

  Output rows 122..127 straddle the two halves; they are computed by extra
  matmuls over 12-row strips (input rows 122..133) of 10 channels stacked
  block-diagonally on 120 partitions (13 groups cover 128 channels).

  ScalarE applies Lrelu(+bias c2) evacuating PSUM -> SBUF bf16; all DMA is
  bf16 with >=512B contiguous runs.  Host pre/post-transposes (not counted
  in NEFF time).
"""

import os
import numpy as np

A = 256
B = 256
R = 32
C = 32
K = 1024
KS = 7
NCORES = 8
P = 128          # channels per core
AO = A - KS + 1  # 250
BO = B - KS + 1  # 250
HP = 128         # rows per half
NJ = HP - KS + 1         # 122 output rows per half per matmul
NB = AO - 2 * NJ         # 6 boundary output rows (122..127)
NBIN = NB + KS - 1       # 12 boundary input rows (122..133)
GCH = 10                 # channels per boundary group
NGRP = (P + GCH - 1) // GCH   # 13 boundary groups (12x10 + 1x8)
G = 8                    # channels per main pipeline group
NG = P // G              # 16 main groups

_COMPILED = {}
LAST_RESULTS = None  # BassKernelResults of the most recent run (for test.py)


def _build_nc():
    import concourse.bacc as bacc
    import concourse.mybir as mybir
    import concourse.tile as tile

    f32 = mybir.dt.float32
    bf16 = mybir.dt.bfloat16
    nc = bacc.Bacc("TRN2", target_bir_lowering=False, debug=False, num_devices=NCORES)

    x_d = nc.declare_dram_parameter("x", [HP, P, 2, B], bf16, isOutput=False)
    w_d = nc.declare_dram_parameter("w", [HP, P, KS, NJ], bf16, isOutput=False)
    xb_d = nc.declare_dram_parameter("xb", [GCH * NBIN, NGRP, B], bf16, isOutput=False)
    wb_d = nc.declare_dram_parameter(
        "wb", [GCH * NBIN, NGRP, KS, GCH * NB], bf16, isOutput=False
    )
    c2_d = nc.declare_dram_parameter("c2", [HP, P], f32, isOutput=False)
    cb_d = nc.declare_dram_parameter("cb", [GCH * NB, NGRP], f32, isOutput=False)
    out_d = nc.declare_dram_parameter("out", [NJ, P, 2, BO], bf16, isOutput=True)
    outb_d = nc.declare_dram_parameter("outb", [GCH * NB, NGRP, BO], bf16, isOutput=True)

    ALPHA = 0.01 / 0.9
    LRELU = mybir.ActivationFunctionType.Lrelu

    with tile.TileContext(nc) as tc:
        from contextlib import ExitStack

        with ExitStack() as ctx:
            const = ctx.enter_context(tc.tile_pool(name="const", bufs=1))
            xpool = ctx.enter_context(tc.tile_pool(name="x", bufs=3))
            wpool = ctx.enter_context(tc.tile_pool(name="w", bufs=3))
            opool = ctx.enter_context(tc.tile_pool(name="o", bufs=3))
            ppool = ctx.enter_context(tc.tile_pool(name="ps", bufs=6, space="PSUM"))
            pbpool = ctx.enter_context(tc.tile_pool(name="psb", bufs=2, space="PSUM"))

            # --- prefetch group 0/1 inputs before the constants so PE can
            # start as early as possible (single in-order DMA queue).
            xs_t = []
            ws_t = []
            for g in range(2):
                xs = xpool.tile([HP, G, 2, B], bf16, tag="xs")
                nc.sync.dma_start(xs[:], x_d[:, g * G : (g + 1) * G, :, :])
                ws = wpool.tile([HP, G, KS, NJ], bf16, tag="ws")
                nc.sync.dma_start(ws[:], w_d[:, g * G : (g + 1) * G, :, :])
                xs_t.append(xs)
                ws_t.append(ws)

            c2_sb = const.tile([HP, P], f32)
            nc.sync.dma_start(c2_sb[:], c2_d[:])
            xb_sb = const.tile([GCH * NBIN, NGRP, B], bf16)
            nc.sync.dma_start(xb_sb[:], xb_d[:])
            wb_sb = const.tile([GCH * NBIN, NGRP, KS, GCH * NB], bf16)
            nc.sync.dma_start(wb_sb[:], wb_d[:])
            cb_sb = const.tile([GCH * NB, NGRP], f32)
            nc.sync.dma_start(cb_sb[:], cb_d[:])
            # boundary outputs accumulate into one tile, DMA'd once at the end
            ob_sb = const.tile([GCH * NB, NGRP, BO], bf16)

            for g in range(NG):
                if g < 2:
                    xs, ws = xs_t[g], ws_t[g]
                else:
                    xs = xpool.tile([HP, G, 2, B], bf16, tag="xs")
                    nc.sync.dma_start(xs[:], x_d[:, g * G : (g + 1) * G, :, :])
                    ws = wpool.tile([HP, G, KS, NJ], bf16, tag="ws")
                    nc.sync.dma_start(ws[:], w_d[:, g * G : (g + 1) * G, :, :])
                outs = opool.tile([HP, G, 2, BO], bf16, tag="outs")

                for c in range(G):
                    ch = g * G + c
                    ps = ppool.tile([HP, 2, B], f32, tag="ps")
                    for dj in range(KS):
                        nc.tensor.matmul(
                            ps[0:NJ, :, 0:BO],
                            ws[:, c, dj, :],
                            xs[:, c, :, dj : dj + BO],
                            start=(dj == 0),
                            stop=(dj == KS - 1),
                        )
                    nc.scalar.activation(
                        outs[0:NJ, c, :, :],
                        ps[0:NJ, :, 0:BO],
                        LRELU,
                        bias=c2_sb[0:NJ, ch : ch + 1],
                        scale=1.0,
                        alpha=ALPHA,
                    )
                nc.sync.dma_start(
                    out_d[:, g * G : (g + 1) * G, :, :], outs[0:NJ, :, :, :]
                )

                # interleave one boundary group per main group (g=2..14)
                s = g - 2
                if 0 <= s < NGRP:
                    nch = GCH if s < NGRP - 1 else P - GCH * (NGRP - 1)
                    npart = nch * NBIN
                    ncol = nch * NB
                    psb = pbpool.tile([GCH * NB, 2, B], f32, tag="psb")
                    for dj in range(KS):
                        nc.tensor.matmul(
                            psb[0:ncol, 0, 0:BO],
                            wb_sb[0:npart, s, dj, 0:ncol],
                            xb_sb[0:npart, s, dj : dj + BO],
                            start=(dj == 0),
                            stop=(dj == KS - 1),
                        )
                    nc.scalar.activation(
                        ob_sb[0:ncol, s, :],
                        psb[0:ncol, 0, 0:BO],
                        LRELU,
                        bias=cb_sb[0:ncol, s : s + 1],
                        scale=1.0,
                        alpha=ALPHA,
                    )

            nc.sync.dma_start(outb_d[:], ob_sb[:])

    nc.compile()
    return nc


def _prep_weights(w_p, b_p, v, g):
    v = v.astype(np.float32)
    v_norm = np.sqrt((v * v).sum(axis=(1, 2), keepdims=True))
    w_eff = g[:, None, None].astype(np.float32) * v / v_norm          # [K,7,7]
    w2 = 0.9 * (1.0 + w_p)[:, None, None].astype(np.float32) * w_eff  # [K,7,7]
    c2 = 0.9 * b_p.astype(np.float32) * w_eff.sum(axis=(1, 2))        # [K]
    return w2, c2


def kernel(x, w_p, b_p, v, g):
    global LAST_RESULTS
    import ml_dtypes
    from concourse.bass_utils import run_bass_kernel_spmd

    bf = ml_dtypes.bfloat16
    x = np.asarray(x, dtype=np.float32)
    w2, c2 = _prep_weights(
        np.asarray(w_p, np.float32),
        np.asarray(b_p, np.float32),
        np.asarray(v, np.float32),
        np.asarray(g, np.float32),
    )

    # channel-major x: [K, A, B], k = r*C + c (matches reference's kernel_index)
    x_t = np.ascontiguousarray(x.transpose(2, 3, 0, 1).reshape(K, A, B))

    jr = np.arange(NJ)
    in_maps = []
    for core in range(NCORES):
        sl = slice(core * P, (core + 1) * P)
        xc = x_t[sl]          # [128, 256, 256] f32
        w2c = w2[sl]          # [128, 7, 7]
        c2c = c2[sl]          # [128]

        # x: [p, ch, h, c];  row = 128*h + p
        xp = np.ascontiguousarray(
            xc.astype(bf).reshape(P, 2, HP, B).transpose(2, 0, 1, 3)
        )
        # banded weights: band[ch, dj, p, j] = w2c[ch, p-j, dj]
        band = np.zeros((P, KS, HP, NJ), np.float32)
        for di in range(KS):
            band[:, :, jr + di, jr] = w2c[:, di, :][:, :, None]
        wp_ = np.ascontiguousarray(band.astype(bf).transpose(2, 0, 1, 3))

        # boundary strips: input rows 122..133 of each channel, 10 per group
        xb = np.zeros((GCH * NBIN, NGRP, B), np.float32)
        strip = xc[:, NJ : NJ + NBIN, :]   # [128, 12, 256], input rows 122..133
        wb = np.zeros((GCH * NBIN, NGRP, KS, GCH * NB), np.float32)
        cb = np.zeros((GCH * NB, NGRP), np.float32)
        for s in range(NGRP):
            nch = GCH if s < NGRP - 1 else P - GCH * (NGRP - 1)
            ch0 = s * GCH
            xb[0 : nch * NBIN, s, :] = strip[ch0 : ch0 + nch].reshape(nch * NBIN, B)
            ar = np.arange(nch)
            for di in range(KS):
                for jj in range(NB):
                    wb[NBIN * ar + jj + di, s, :, NB * ar + jj] = w2c[
                        ch0 : ch0 + nch, di, :
                    ]
            cb[0 : nch * NB, s] = np.repeat(c2c[ch0 : ch0 + nch], NB)

        in_maps.append(
            {
                "x": xp,
                "w": wp_,
                "xb": xb.astype(bf),
                "wb": wb.astype(bf),
                "c2": np.ascontiguousarray(
                    np.broadcast_to(c2c[None, :], (HP, P))
                ),
                "cb": cb,
            }
        )

    key = "v2_banded"
    if key not in _COMPILED:
        _COMPILED[key] = _build_nc()
    nc = _COMPILED[key]

    trace = os.environ.get("KRN_TRACE", "0") == "1"
    res = run_bass_kernel_spmd(nc, in_maps, list(range(NCORES)), trace=trace)
    LAST_RESULTS = res

    out_full = np.empty((K, AO, BO), dtype=np.float32)
    for core in range(NCORES):
        od = np.asarray(res.results[core]["out"]).astype(np.float32)
        ob = np.asarray(res.results[core]["outb"]).astype(np.float32)
        oc = out_full[core * P : (core + 1) * P]
        t = od.transpose(1, 2, 0, 3)              # [ch, h, j, c]
        oc[:, 0:NJ] = t[:, 0]
        oc[:, HP : HP + NJ] = t[:, 1]
        # ob[nb*ci+jj, s, c] -> channel GCH*s+ci, row NJ+jj (122..127)
        obt = ob.reshape(GCH, NB, NGRP, BO).transpose(2, 0, 1, 3).reshape(
            GCH * NGRP, NB, BO
        )
        oc[:, NJ : NJ + NB] = obt[0:P]
        out_full[core * P : (core + 1) * P] = oc

    # [K, AO, BO] -> [AO, BO, R, C]
    return np.ascontiguousarray(
        out_full.reshape(R, C, AO, BO).transpose(2, 3, 0, 1)
    )


if __name__ == "__main__":
    rng = np.random.default_rng(0)
    xs = rng.standard_normal((A, B, R, C), dtype=np.float32)
    out = kernel(
        xs,
        rng.standard_normal(K).astype(np.float32) * 0.1,
        rng.standard_normal(K).astype(np.float32) * 0.1,
        rng.standard_normal((K, KS, KS)).astype(np.float32),
        rng.standard_normal(K).astype(np.float32),
    )
    print(out.shape, out.dtype)
